# revision 123
# baseline (speedup 1.0000x reference)
"""Single-head causal attention (B=4, T=4096, E=1024, H=128) on 8 TRN2 cores.

Sharding: 2 cores per batch, "folded triangle" split of the causal work.
Chunk0 = queries [0,2048), chunk1 = [2048,4096).
  core (b, 0): TRI : chunk0 q vs k in [0, q]        (causal triangle)
               RECT: chunk1 q vs k in [0, 1024)     (no mask)
  core (b, 1): TRI : chunk1 q vs k in [2048, q]     (causal triangle)
               RECT: chunk1 q vs k in [1024, 2048)  (no mask)
Both cores run the *identical* program: a 2048-token causal self-attention
triangle plus a 2048q x 1024k rectangle; only the data differs.
Outputs are unnormalized accumulators acc = P@V and row-sums l = P@1.
Host sums partials for chunk1 and normalizes.

Datapath (v4):
  - Projections run as residual-fp8 DoubleRow matmuls at 0.5 cycles/row:
    V uses 3 terms (x8@W8 + xr8@W8 + x8@Wr8, residuals in e5m2, 0.75x
    bf16 cost, numerically equivalent); Q/K drop the W-residual term
    (2 terms, 0.5x cost) - the W-quantization noise lands on S logits
    where the fp8-S error budget has margin. Host ships x8/xr8 packed in
    one fp8 tensor (same bytes as bf16); e5m2 slices are bitcast.
  - Q/K are stored fp8 in DoubleRow layout [64p, 2, cols] (h = 64*g + p);
    the S matmul contracts (p, g) at 0.5 cycles/row. The upper h-half is
    partition-shifted by a GPSIMD SBUF->SBUF copy (no DMA involved).
  - V is projected directly in [tok, h] orientation (x stationary), no
    PE transposes; AV stays bf16 (fp8 V/P fail the tolerance).
  - exp on ScalarE per job pair [128k, 1024q], P in bf16; causal masking
    is multiplicative post-exp via affine_select on the Pool engine.
  - Row-sums l: per-q-block P-accumulation on DVE in bf16 (2x mode),
    deferred one window so the in-order DVE queue (which also feeds the
    PE's projection evacuations) never blocks on the exp chain; a single
    ones-matmul per q-block reduces the 128 partials.
  - Schedule: tri windows are PE/proj-heavy, rect windows exp-chain
    bound; they are interleaved (0,1,4,2,5,3,6,7) with projection
    half-thunks woven between jobs, S issued 2 jobs ahead, outputs
    flushed one window late, and PE p-state warmed by dummy matmuls
    during the initial DMA fill.

Cost-model timeline: ~71 us vs ~116 us for the bf16 v1 baseline.
Measured rel err on hw: 1.73e-2 (limit 2e-2).
"""

import sys

if "/opt/trn_rl_repo" not in sys.path:
    sys.path.insert(0, "/opt/trn_rl_repo")

import numpy as np
import ml_dtypes

import concourse.bacc as bacc
import concourse.bass as bass
import concourse.mybir as mybir
from concourse import masks, tile
from concourse.bass_utils import run_bass_kernel_spmd

E = 1024
H = 128
T = 4096
CH = 2048            # chunk length
TQ = 4096            # q tokens per core: [tri own-chunk 2048 | rect chunk1 2048]
RK = 1024            # rect-k region length
NKT_RK = RK // 128   # 8 k-tiles in the rect-k region
SCALE = 1.0 / np.sqrt(np.float32(H))

F32 = mybir.dt.float32
BF16 = mybir.dt.bfloat16
FP8 = mybir.dt.float8e4
FP8E5 = mybir.dt.float8e5
DR = mybir.MatmulPerfMode.DoubleRow

EC = E // 128        # 8 contraction chunks for the projections
NB_Q = TQ // 512     # 8 q blocks
NB_RK = RK // 512    # 2 rect-k blocks

_CACHED = {}
TRACE = False
TRACE_CORES = None
LAST_RESULTS = None


def _build(loop_n=None):
    nc = bacc.Bacc("TRN2", target_bir_lowering=False, debug=False, num_devices=8)
    # x host-tiled for fp8 DoubleRow projections with e5m2 residual
    # compensation: [128p, tb, ecp(4), sel(x8/xr8), g(2), 512]; e-index
    # = 256*ecp + 128*g + p. sel 1 holds e5m2 BITS in the e4 container.
    xq_in = nc.dram_tensor(
        "xq_in", [128, NB_Q, 4, 2, 2, 512], FP8, kind="ExternalInput").ap()
    xrk_in = nc.dram_tensor(
        "xrk_in", [128, NB_RK, 4, 2, 2, 512], FP8, kind="ExternalInput").ap()
    # weights: [128p, name(k,q,v), sel(W8/Wr8), ecp, g, H]
    w_in = nc.dram_tensor(
        "w_in", [128, 3, 2, 4, 2, H], FP8, kind="ExternalInput").ap()
    acc_out = nc.dram_tensor("acc_out", [H, TQ], F32, kind="ExternalOutput").ap()
    warm_out = nc.dram_tensor("warm_out", [1, 1], F32, kind="ExternalOutput").ap()
    l_out = nc.dram_tensor("l_out", [1, TQ], F32, kind="ExternalOutput").ap()

    import contextlib

    with tile.TileContext(nc) as tc:
        loop_cm = tc.For_i(0, loop_n, 1) if loop_n else contextlib.nullcontext()
        with (
            tc.tile_pool(name="const", bufs=1) as constp,
            tc.tile_pool(name="wpool", bufs=1) as wpool,
            tc.tile_pool(name="xin", bufs=14) as xin,
            tc.tile_pool(name="proj", bufs=1) as projp,
            tc.tile_pool(name="ppool", bufs=24) as ppool,
            tc.tile_pool(name="paccp", bufs=3) as paccp,
            tc.tile_pool(name="outp", bufs=4) as outp,
            tc.tile_pool(name="stgp", bufs=6) as stgp,
            tc.tile_pool(name="psS", bufs=2, space="PSUM") as psS,
            tc.tile_pool(name="psY", bufs=2, space="PSUM") as psY,
            tc.tile_pool(name="psT", bufs=2, space="PSUM") as psT,
            loop_cm,
        ):
            # ---- input DMAs first: wk alone (first projection is K), then
            # the first x half, then the remaining weights - the cost-model
            # DMA device is serialized, so order = earliest-need
            wt = wpool.tile([128, 3, 2, 4, 2, H], FP8, tag="w")
            nc.sync.dma_start(wt[:, 0], w_in[:, 0])
            NI = {"k": 0, "q": 1, "v": 2}

            xb0a = xin.tile([128, 2, 2, 2, 512], FP8, tag="xb", name="xb0a")
            nc.sync.dma_start(xb0a[:], xq_in[:, 0, :2])
            xb0b = xin.tile([128, 2, 2, 2, 512], FP8, tag="xb", name="xb0b")
            nc.sync.dma_start(xb0b[:], xq_in[:, 0, 2:])
            nc.sync.dma_start(wt[:, 1], w_in[:, 1])   # Wq before Wv: the
            nc.sync.dma_start(wt[:, 2], w_in[:, 2])   # q-proj needs it first

            # ---- constants ----
            # warm-up source first: the PE dummies depend only on this
            wsrc = constp.tile([128, 512], BF16, tag="wsrc")
            nc.gpsimd.memset(wsrc[:], 0.0)
            wps = psT.tile([128, 512], F32, tag="psproj", name="warmps")
            for r in range(2):
                nc.tensor.matmul(wps[:1, :], wsrc[:, :1], wsrc[:],
                                 start=(r == 0), stop=(r == 1))

            ones_f = constp.tile([128, 1], F32, tag="ones32")
            nc.gpsimd.memset(ones_f[:], 1.0)
            ones = constp.tile([128, 1], BF16, tag="ones")
            nc.vector.tensor_copy(ones[:], ones_f[:])
            # multiplicative diag pattern p: [128k, 512q] bf16,
            # 1 where q >= k + 128p else 0 (keep-mask, applied post-exp)
            diag = []
            for p in range(4):
                dm = constp.tile([128, 512], BF16, tag=f"diag{p}")
                nc.gpsimd.memset(dm[:], 1.0)
                nc.gpsimd.affine_select(
                    out=dm[:], in_=dm[:],
                    compare_op=mybir.AluOpType.is_ge,
                    fill=0.0, base=-128 * p,
                    pattern=[[1, 512]], channel_multiplier=-1,
                )
                diag.append(dm)

            warm = constp.tile([1, 1], F32, tag="warm")
            nc.scalar.activation(
                warm[:], ones_f[:1, :1], mybir.ActivationFunctionType.Exp, scale=1.0
            )
            nc.sync.dma_start(warm_out, warm[:])

            # ---- projection targets ----
            # Q/K in fp8 DoubleRow layout [64p, 2, cols]: h = 64*g + p for
            # DR group g - the S matmul contracts (p, g) pairs at 0.5
            # cycles/row. V [k-tiles, h] stays bf16 (fp8 V fails tolerance).
            # kv tile space: tiles 0..7 = rect-k, 8..23 = tri chunk
            qt = projp.tile([64, 2, TQ], FP8, tag="qt")
            kt = projp.tile([64, 2, RK + CH], FP8, tag="kt")
            vsb = projp.tile([128, NKT_RK + CH // 128, 128], BF16, tag="v")
            l_sb = projp.tile([1, TQ], F32, tag="lsb")

            xblocks = {("q", 0): (xb0a, xb0b)}

            def get_xb(src, key, tb):
                """x block loaded as two half tiles, each in two quarter
                DMAs - small transfers so the fp8-shift DMAs are never
                stuck behind a long one."""
                if key in xblocks:
                    return xblocks[key]
                xa = xin.tile([128, 2, 2, 2, 512], FP8, tag="xb")
                nc.sync.dma_start(xa[:], src[:, tb, :2])
                xb = xin.tile([128, 2, 2, 2, 512], FP8, tag="xb")
                nc.sync.dma_start(xb[:], src[:, tb, 2:])
                xblocks[key] = (xa, xb)
                return xblocks[key]

            _stg_n = [0]

            def stg_alt():
                _stg_n[0] += 1
                return _stg_n[0] % 2 == 0

            # 3-term residual fp8 projection: x@W ~= x8@W8 + xr8@W8 + x8@Wr8
            # (residuals in e5m2; the dropped xr8@Wr8 term is ~0.2%)
            TERMS = ((0, 0), (1, 0), (0, 1))  # (x-sel, w-sel)

            def xsl(xpair, h, pi, xs):
                """[128, 2(g), 512] x slice: ecp = 2h + pi."""
                a = xpair[h][:, pi, xs]
                return a.bitcast(FP8E5) if xs else a

            def wsl(name, ws, ecp):
                """[128, 2(g), H] weight slice."""
                a = wt[:, NI[name], ws, ecp]
                return a.bitcast(FP8E5) if ws else a

            # proj work is emitted in half-units so it can be woven
            # finely between attention jobs
            def proj_qk(name, xpair, dcol):
                """One 512-token q/k projection (3-term fp8 DR) as two
                half-thunks (ecp 0-1 / 2-3, 6 DR matmuls each).

                Evacuation to the fp8 DR layout: h 0..63 copies straight
                (same partitions); h 64..127 converts into a staging tile
                and a small SBUF->SBUF DMA shifts it down to partitions
                0..63, group 1; the two half-conversions run on different
                engines in parallel so the psum slot frees sooner.
                """
                dst = qt if name == "q" else kt
                ps = psT.tile([128, 512], F32, tag="psproj")

                def half(h):
                    # Q/K drop the x8@Wr8 term (2-term): the W-quantization
                    # noise lands on S logits where the error budget has
                    # margin; V keeps all 3 terms (it is tolerance-critical)
                    terms = TERMS[:2]
                    for ti, (xs, ws) in enumerate(terms):
                        for pi in range(2):
                            nc.tensor.matmul(
                                ps[:], wsl(name, ws, 2 * h + pi),
                                xsl(xpair, h, pi, xs),
                                start=(h == 0 and ti == 0 and pi == 0),
                                stop=(h == 1 and ti == len(terms) - 1
                                      and pi == 1),
                                perf_mode=DR,
                            )
                    if h == 1:
                        nc.vector.tensor_copy(
                            dst[:, 0, dcol : dcol + 512], ps[:64, :]
                        )
                        stg = stgp.tile([128, 512], FP8, tag="stg")
                        nc.vector.tensor_copy(stg[64:, :], ps[64:, :])
                        nc.gpsimd.tensor_copy(
                            dst[:, 1, dcol : dcol + 512], stg[64:, :]
                        )

                yield lambda: half(0)
                yield lambda: half(1)

            def proj_v(xpair, kv0):
                """One 512-token V projection directly as [tok, h] tiles,
                3-term fp8 DR, as two half-thunks (2 token-tiles each).

                x slice is stationary, Wv is moving: out[tok, h] per
                128-token tile; 4 tiles side by side in one psum bank,
                one evac into vsb[:, kv0:kv0+4, :].
                """
                ps = psT.tile([128, 512], F32, tag="psproj")

                def half(hh):
                    for t in range(2 * hh, 2 * hh + 2):
                        dst = ps[:, 128 * t : 128 * (t + 1)]
                        mm = 0
                        for ti, (xs, ws) in enumerate(TERMS):
                            for h in range(2):
                                for pi in range(2):
                                    nc.tensor.matmul(
                                        dst,
                                        xsl(xpair, h, pi, xs)[
                                            :, :, 128 * t : 128 * (t + 1)],
                                        wsl("v", ws, 2 * h + pi),
                                        start=(mm == 0), stop=(mm == 11),
                                        perf_mode=DR,
                                    )
                                    mm += 1
                    if hh == 1:
                        # V evac on Act: it is idle during the proj-heavy
                        # phase while DVE is the loaded engine there
                        nc.scalar.copy(
                            vsb[:, kv0 // 128 : kv0 // 128 + 4, :], ps[:]
                        )

                yield lambda: half(0)
                yield lambda: half(1)

            def thunk_kvq(b):
                def f():
                    xp = get_xb(xq_in, ("q", b), b)
                    yield from proj_qk("k", xp, RK + 512 * b)
                    yield from proj_qk("q", xp, 512 * b)
                    yield from proj_v(xp, RK + 512 * b)
                return f

            def thunk_kq(b):
                def f():
                    xp = get_xb(xq_in, ("q", b), b)
                    yield from proj_qk("k", xp, RK + 512 * b)
                    yield from proj_qk("q", xp, 512 * b)
                return f

            def thunk_vq(b):
                def f():
                    xp = get_xb(xq_in, ("q", b), b)
                    yield from proj_v(xp, RK + 512 * b)
                return f

            def thunk_rk(b):
                def f():
                    xp = get_xb(xrk_in, ("rk", b), b)
                    yield from proj_qk("k", xp, 512 * b)
                    yield from proj_v(xp, 512 * b)
                return f

            def thunk_q(b):
                def f():
                    xp = get_xb(xq_in, ("q", b), b)
                    yield from proj_qk("q", xp, 512 * b)
                return f

            # ---- attention job list (same folded-triangle pairing as v1) ----
            # job = (qb, (k0, d0), (k1, d1), first_in_block, last_in_block)
            # d = diag position (0..3) for tiles on the causal diagonal; the
            # AV matmul for such tiles only covers q-columns [128*d, 512).
            jobs = []
            for qb in range(4):  # triangle over kv tiles 8..(8+4qb+4)
                kts = []
                for j in range(4 * qb + 4):
                    dp = j - 4 * qb if j >= 4 * qb else None
                    kts.append((NKT_RK + j, dp))
                for i in range(len(kts) // 2):
                    jobs.append((qb, kts[2 * i], kts[2 * i + 1],
                                 i == 0, 2 * i + 2 == len(kts)))
            for qb in range(4, 8):  # rect over kv tiles 0..7
                for i in range(NKT_RK // 2):
                    jobs.append((qb, (2 * i, None), (2 * i + 1, None),
                                 i == 0, 2 * i + 2 == NKT_RK))

            n = len(jobs)
            ss_t = [None] * n
            pt_t = [None] * n
            ybank = {}
            pacc = {}
            pending_out = []

            def s_stage(j):
                qb, (k0, d0), (k1, d1), _, _ = jobs[j]
                ss = psS.tile([128, 1024], F32, tag="s")
                ss_t[j] = ss
                qs = qt[:, :, 512 * qb : 512 * (qb + 1)]
                nc.tensor.matmul(ss[:, :512], kt[:, :, 128 * k0 : 128 * (k0 + 1)],
                                 qs, start=True, stop=True,
                                 perf_mode=mybir.MatmulPerfMode.DoubleRow)
                nc.tensor.matmul(ss[:, 512:], kt[:, :, 128 * k1 : 128 * (k1 + 1)],
                                 qs, start=True, stop=True,
                                 perf_mode=mybir.MatmulPerfMode.DoubleRow)
                pt = ppool.tile([128, 1024], BF16, tag="pt")
                pt_t[j] = pt
                nc.scalar.activation(
                    pt[:], ss[:], mybir.ActivationFunctionType.Exp, scale=SCALE
                )

            def av_stage(j):
                qb, (k0, d0), (k1, d1), first, last = jobs[j]
                pt = pt_t[j]
                # causal mask on the diagonal tiles: zero the upper-triangle
                # region of pt in place on the (otherwise idle) Pool engine,
                # keeping DVE free for the P-accumulation
                if d0 is not None:
                    nc.gpsimd.affine_select(
                        out=pt[:, :512], in_=pt[:, :512],
                        compare_op=mybir.AluOpType.is_ge,
                        fill=0.0, base=-128 * d0,
                        pattern=[[1, 512]], channel_multiplier=-1,
                    )
                if d1 is not None:
                    nc.gpsimd.affine_select(
                        out=pt[:, 512:], in_=pt[:, 512:],
                        compare_op=mybir.AluOpType.is_ge,
                        fill=0.0, base=-128 * d1,
                        pattern=[[1, 512]], channel_multiplier=-1,
                    )
                if first:
                    ys = psY.tile([128, 512], F32, tag="y", name=f"ys{qb}")
                    ybank[qb] = ys
                else:
                    ys = ybank[qb]
                # diagonal tiles only contribute to q >= 128*d: shrink the
                # AV matmul to the live columns (mask already zeroed the rest
                # of pt for the P-accumulation)
                o0 = 128 * d0 if d0 else 0
                o1 = 128 * d1 if d1 else 0
                nc.tensor.matmul(ys[:, o0:], vsb[:, k0, :], pt[:, o0:512],
                                 start=first, stop=False)
                nc.tensor.matmul(ys[:, o1:], vsb[:, k1, :], pt[:, 512 + o1 :],
                                 start=False, stop=last)
                if last:
                    pending_out.append(qb)

            def pacc_ops(qb):
                """P-accumulation ops for a finished q-block, deferred to
                the next window: all pt inputs are long done, so these DVE
                adds never block the in-order DVE queue (whose later entries
                feed the PE's projection pipeline). Returned as thunks so
                the window can spread them out instead of bursting."""
                ops = []

                def first(j):
                    pa = paccp.tile([128, 512], BF16, tag="pacc",
                                    name=f"pacc{qb}")
                    pacc[qb] = pa
                    nc.vector.tensor_add(pa[:], pt_t[j][:, :512],
                                         pt_t[j][:, 512:])

                def rest(j, half):
                    pa = pacc[qb]
                    nc.vector.tensor_add(pa[:], pa[:],
                                         pt_t[j][:, 512 * half : 512 * half + 512])

                for i, j in enumerate(by_qb[qb]):
                    if i == 0:
                        ops.append(lambda j=j: first(j))
                    else:
                        ops.append(lambda j=j: rest(j, 0))
                        ops.append(lambda j=j: rest(j, 1))
                return ops

            def flush_out():
                """Row-sum + output DMA for a finished q-block. Deferred to
                the next window so the PE's ones-matmul never waits on the
                tail of the DVE P-accumulation chain; outputs DMA straight
                from PSUM (no DVE evacuation)."""
                while pending_out:
                    qb = pending_out.pop(0)
                    ys, pa = ybank[qb], pacc[qb]
                    ls = psT.tile([128, 512], F32, tag="psproj", name=f"ls{qb}")
                    nc.tensor.matmul(ls[:1, :], ones[:], pa[:],
                                     start=True, stop=True)
                    yo = outp.tile([128, 512], F32, tag="yo")
                    # the very last q-block evacuates on Act (idle by then,
                    # and parallel with DVE's inline P-accumulation tail)
                    cp = nc.scalar.copy if qb == worder[-1] else \
                        nc.vector.tensor_copy
                    cp(yo[:], ys[:])
                    nc.sync.dma_start(acc_out[:, 512 * qb : 512 * (qb + 1)], yo[:])
                    # l rows collect in SBUF; two DMAs total (the HWDGE
                    # queue charges a fixed cost per DMA)
                    cp(l_sb[:, 512 * qb : 512 * (qb + 1)], ls[:1, :])

            # ---- interleaved schedule ----
            # per-qb window: list of proj thunks to weave between that
            # window's jobs (deps: kv(b) ready before qb=b jobs; rk before
            # qb4; q(b) before qb=b / qb=4+b jobs)
            # window order interleaves rect q-blocks into the proj-heavy tri
            # phase: rect jobs are exp-chain-bound, tri windows are PE-bound
            # with Act idle - blending them keeps both engines fed
            worder = [0, 1, 4, 2, 5, 3, 6, 7]
            pre = [thunk_kvq(0)]
            weave = {
                0: [thunk_kvq(1)],
                1: [thunk_rk(0), thunk_rk(1), thunk_q(4)],
                4: [thunk_kq(2)],
                2: [thunk_vq(2), thunk_q(5)],
                5: [thunk_kq(3)],
                3: [thunk_vq(3), thunk_q(6)],
                6: [thunk_q(7)],
                7: [],
            }

            def expand(thunks):
                ops = []
                for t in thunks:
                    ops.extend(list(t()))
                return ops

            # pre-phase: k/q of block 0, then the first two S stages
            # (they do not need V), then V - the exp chain starts ~1.7us
            # earlier
            pre_ops = expand(pre)
            for op in pre_ops[:4]:
                op()

            # group jobs by qb; execution sequence follows worder
            by_qb = {}
            for j, jb in enumerate(jobs):
                by_qb.setdefault(jb[0], []).append(j)
            jseq = [j for qb in worder for j in by_qb[qb]]
            pos_of = {j: i for i, j in enumerate(jseq)}

            def emit_window(qb, prev_qb, inline_pacc=False):
                js = by_qb[qb]
                ops = expand(weave[qb])
                nj = len(js)
                done = 0
                pac = pacc_ops(prev_qb) if prev_qb is not None else []
                pdone = 0
                for i, j in enumerate(js):
                    # weave proj ops evenly, finishing early: the
                    # lookahead-2 S of the next window needs them. Proj ops
                    # go BEFORE the pacc adds so their DVE evacuations are
                    # never stuck behind the (never-blocking but bulky)
                    # accumulation queue.
                    want = len(ops)
                    while done < want:
                        ops[done]()
                        done += 1
                    # spread the previous q-block's P-accumulation adds
                    # across the window; all done before the flush at the
                    # last iteration (psY liveness allows that deferral)
                    pwant = min(len(pac), (i + 4) * len(pac) // nj)
                    while pdone < pwant:
                        pac[pdone]()
                        pdone += 1
                    if i == nj - 1:
                        flush_out()  # previous q-block's row-sum + out DMA
                    # issue S two jobs ahead so an exp-lagged AV never
                    # leaves the PE without queued work
                    p2 = pos_of[j] + 2
                    if p2 < len(jseq):
                        s_stage(jseq[p2])
                    av_stage(j)
                    if inline_pacc:
                        # last window: accumulate as we go so the program
                        # tail is not serialized behind a deferred chain
                        pt = pt_t[j]
                        if i == 0:
                            pa = paccp.tile([128, 512], BF16, tag="pacc",
                                            name=f"pacc{qb}")
                            pacc[qb] = pa
                            nc.vector.tensor_add(pa[:], pt[:, :512], pt[:, 512:])
                        else:
                            pa = pacc[qb]
                            nc.vector.tensor_add(pa[:], pa[:], pt[:, :512])
                            nc.vector.tensor_add(pa[:], pa[:], pt[:, 512:])

            s_stage(jseq[0])
            s_stage(jseq[1])
            for op in pre_ops[4:]:
                op()
            prev = None
            for wi, qb in enumerate(worder):
                emit_window(qb, prev, inline_pacc=(wi == len(worder) - 1))
                if wi == len(worder) - 2:
                    # ship the first 6 q-blocks' row-sums before the tail
                    nc.sync.dma_start(l_out[:, : 512 * 6],
                                      l_sb[:, : 512 * 6])
                prev = qb
            flush_out()
            nc.sync.dma_start(l_out[:, 512 * 6 :], l_sb[:, 512 * 6 :])

    nc.compile()
    return nc


def _prep_x(xpart):
    """[Tpart, E] f32 -> fp8 3-term layout [128, tb, ecp, sel, g, 512].

    e-index = 256*ecp + 128*g + p. sel 0 = e4m3(x), sel 1 = e5m2(x - x8)
    (e5m2 bits shipped in the e4m3 container; the device bitcasts).
    """
    tb = xpart.shape[0] // 512
    a = xpart.T.astype(np.float32).reshape(4, 2, 128, tb, 512)
    x8 = np.asarray(a, dtype=ml_dtypes.float8_e4m3)
    xr8 = np.asarray(a - np.asarray(x8, np.float32),
                     dtype=ml_dtypes.float8_e5m2)
    out = np.empty((128, tb, 4, 2, 2, 512), np.uint8)
    out[:, :, :, 0] = x8.view(np.uint8).transpose(2, 3, 0, 1, 4)
    out[:, :, :, 1] = xr8.view(np.uint8).transpose(2, 3, 0, 1, 4)
    return np.ascontiguousarray(out).view(ml_dtypes.float8_e4m3)


def _prep_w(w):
    """[H, E] f32 -> fp8 [128, sel, ecp, g, H] (W8 + e5m2 residual)."""
    a = w.T.astype(np.float32).reshape(4, 2, 128, H)
    w8 = np.asarray(a, dtype=ml_dtypes.float8_e4m3)
    wr8 = np.asarray(a - np.asarray(w8, np.float32),
                     dtype=ml_dtypes.float8_e5m2)
    out = np.empty((128, 2, 4, 2, H), np.uint8)
    out[:, 0] = w8.view(np.uint8).transpose(2, 0, 1, 3)
    out[:, 1] = wr8.view(np.uint8).transpose(2, 0, 1, 3)
    return out


def kernel(x_in, Wq, Wk, Wv):
    B, T_, E_ = x_in.shape
    assert (B, T_, E_) == (4, T, E)
    nc = _CACHED.get("nc")
    if nc is None:
        nc = _CACHED["nc"] = _build()

    # weights in one tensor, order (k, q, v) to match the device layout
    w = np.ascontiguousarray(
        np.stack([_prep_w(Wk), _prep_w(Wq), _prep_w(Wv)], axis=1)
    ).view(ml_dtypes.float8_e4m3)
    in_maps = []
    for c in range(8):
        b, h = c // 2, c % 2
        xb = np.asarray(x_in[b], dtype=np.float32)
        c0, c1 = xb[:CH], xb[CH:]
        own = c0 if h == 0 else c1
        xq = np.concatenate([own, c1], axis=0)        # [4096, E]
        rk = xb[0:RK] if h == 0 else xb[RK : 2 * RK]  # [1024, E]
        in_maps.append(
            {"xq_in": _prep_x(xq), "xrk_in": _prep_x(rk), "w_in": w}
        )

    kw = {}
    if TRACE:
        kw = {"trace": True, "trace_cores": TRACE_CORES}
    res = run_bass_kernel_spmd(nc, in_maps, core_ids=list(range(8)), **kw)
    global LAST_RESULTS
    LAST_RESULTS = res

    y = np.empty((B, T, H), dtype=np.float32)
    for b in range(4):
        r0, r1 = res.results[2 * b], res.results[2 * b + 1]
        a0, l0 = r0["acc_out"], r0["l_out"][0]
        a1, l1 = r1["acc_out"], r1["l_out"][0]
        y[b, :CH] = (a0[:, :CH] / l0[:CH]).T
        acc = a0[:, CH:] + a1[:, :CH] + a1[:, CH:]
        l = l0[CH:] + l1[:CH] + l1[CH:]
        y[b, CH:] = (acc / l).T
    return y


# revision 125
# speedup vs baseline: 1.0013x; 1.0013x over previous
"""Single-head causal attention (B=4, T=4096, E=1024, H=128) on 8 TRN2 cores.

Sharding: 2 cores per batch, "folded triangle" split of the causal work.
Chunk0 = queries [0,2048), chunk1 = [2048,4096).
  core (b, 0): TRI : chunk0 q vs k in [0, q]        (causal triangle)
               RECT: chunk1 q vs k in [0, 1024)     (no mask)
  core (b, 1): TRI : chunk1 q vs k in [2048, q]     (causal triangle)
               RECT: chunk1 q vs k in [1024, 2048)  (no mask)
Both cores run the *identical* program: a 2048-token causal self-attention
triangle plus a 2048q x 1024k rectangle; only the data differs.
Outputs are unnormalized accumulators acc = P@V and row-sums l = P@1.
Host sums partials for chunk1 and normalizes.

Datapath (v4):
  - Projections run as residual-fp8 DoubleRow matmuls at 0.5 cycles/row:
    V uses 3 terms (x8@W8 + xr8@W8 + x8@Wr8, residuals in e5m2, 0.75x
    bf16 cost, numerically equivalent); Q/K drop the W-residual term
    (2 terms, 0.5x cost) - the W-quantization noise lands on S logits
    where the fp8-S error budget has margin. Host ships x8/xr8 packed in
    one fp8 tensor (same bytes as bf16); e5m2 slices are bitcast.
  - Q/K are stored fp8 in DoubleRow layout [64p, 2, cols] (h = 64*g + p);
    the S matmul contracts (p, g) at 0.5 cycles/row. The upper h-half is
    partition-shifted by a GPSIMD SBUF->SBUF copy (no DMA involved).
  - V is projected directly in [tok, h] orientation (x stationary), no
    PE transposes; AV stays bf16 (fp8 V/P fail the tolerance).
  - exp on ScalarE per job pair [128k, 1024q], P in bf16; causal masking
    is multiplicative post-exp via affine_select on the Pool engine.
  - Row-sums l: per-q-block P-accumulation on DVE in bf16 (2x mode),
    deferred one window so the in-order DVE queue (which also feeds the
    PE's projection evacuations) never blocks on the exp chain; a single
    ones-matmul per q-block reduces the 128 partials.
  - Schedule: tri windows are PE/proj-heavy, rect windows exp-chain
    bound; they are interleaved (0,1,4,2,5,3,6,7) with projection
    half-thunks woven between jobs, S issued 2 jobs ahead, outputs
    flushed one window late, and PE p-state warmed by dummy matmuls
    during the initial DMA fill.

Cost-model timeline: ~71 us vs ~116 us for the bf16 v1 baseline.
Measured rel err on hw: 1.73e-2 (limit 2e-2).
"""

import sys

if "/opt/trn_rl_repo" not in sys.path:
    sys.path.insert(0, "/opt/trn_rl_repo")

import numpy as np
import ml_dtypes

import concourse.bacc as bacc
import concourse.bass as bass
import concourse.mybir as mybir
from concourse import masks, tile
from concourse.bass_utils import run_bass_kernel_spmd

E = 1024
H = 128
T = 4096
CH = 2048            # chunk length
TQ = 4096            # q tokens per core: [tri own-chunk 2048 | rect chunk1 2048]
RK = 1024            # rect-k region length
NKT_RK = RK // 128   # 8 k-tiles in the rect-k region
SCALE = 1.0 / np.sqrt(np.float32(H))

F32 = mybir.dt.float32
BF16 = mybir.dt.bfloat16
FP8 = mybir.dt.float8e4
FP8E5 = mybir.dt.float8e5
DR = mybir.MatmulPerfMode.DoubleRow

EC = E // 128        # 8 contraction chunks for the projections
NB_Q = TQ // 512     # 8 q blocks
NB_RK = RK // 512    # 2 rect-k blocks

_CACHED = {}
TRACE = False
TRACE_CORES = None
LAST_RESULTS = None


def _build(loop_n=None):
    nc = bacc.Bacc("TRN2", target_bir_lowering=False, debug=False, num_devices=8)
    # x host-tiled for fp8 DoubleRow projections with e5m2 residual
    # compensation: [128p, tb, ecp(4), sel(x8/xr8), g(2), 512]; e-index
    # = 256*ecp + 128*g + p. sel 1 holds e5m2 BITS in the e4 container.
    xq_in = nc.dram_tensor(
        "xq_in", [128, NB_Q, 4, 2, 2, 512], FP8, kind="ExternalInput").ap()
    xrk_in = nc.dram_tensor(
        "xrk_in", [128, NB_RK, 4, 2, 2, 512], FP8, kind="ExternalInput").ap()
    # weights: [128p, name(k,q,v), sel(W8/Wr8), ecp, g, H]
    w_in = nc.dram_tensor(
        "w_in", [128, 3, 2, 4, 2, H], FP8, kind="ExternalInput").ap()
    acc_out = nc.dram_tensor("acc_out", [H, TQ], F32, kind="ExternalOutput").ap()
    warm_out = nc.dram_tensor("warm_out", [1, 1], F32, kind="ExternalOutput").ap()
    l_out = nc.dram_tensor("l_out", [1, TQ], F32, kind="ExternalOutput").ap()

    import contextlib

    with tile.TileContext(nc) as tc:
        loop_cm = tc.For_i(0, loop_n, 1) if loop_n else contextlib.nullcontext()
        with (
            tc.tile_pool(name="const", bufs=1) as constp,
            tc.tile_pool(name="wpool", bufs=1) as wpool,
            tc.tile_pool(name="xin", bufs=14) as xin,
            tc.tile_pool(name="proj", bufs=1) as projp,
            tc.tile_pool(name="ppool", bufs=24) as ppool,
            tc.tile_pool(name="paccp", bufs=4) as paccp,
            tc.tile_pool(name="outp", bufs=4) as outp,
            tc.tile_pool(name="stgp", bufs=8) as stgp,
            tc.tile_pool(name="psS", bufs=2, space="PSUM") as psS,
            tc.tile_pool(name="psY", bufs=2, space="PSUM") as psY,
            tc.tile_pool(name="psT", bufs=2, space="PSUM") as psT,
            loop_cm,
        ):
            # ---- input DMAs first: wk alone (first projection is K), then
            # the first x half, then the remaining weights - the cost-model
            # DMA device is serialized, so order = earliest-need
            wt = wpool.tile([128, 3, 2, 4, 2, H], FP8, tag="w")
            nc.sync.dma_start(wt[:, 0], w_in[:, 0])
            NI = {"k": 0, "q": 1, "v": 2}

            xb0a = xin.tile([128, 2, 2, 2, 512], FP8, tag="xb", name="xb0a")
            nc.sync.dma_start(xb0a[:], xq_in[:, 0, :2])
            xb0b = xin.tile([128, 2, 2, 2, 512], FP8, tag="xb", name="xb0b")
            nc.sync.dma_start(xb0b[:], xq_in[:, 0, 2:])
            nc.sync.dma_start(wt[:, 1], w_in[:, 1])   # Wq before Wv: the
            nc.sync.dma_start(wt[:, 2], w_in[:, 2])   # q-proj needs it first

            # ---- constants ----
            # warm-up source first: the PE dummies depend only on this
            wsrc = constp.tile([128, 512], BF16, tag="wsrc")
            nc.gpsimd.memset(wsrc[:], 0.0)
            wps = psT.tile([128, 512], F32, tag="psproj", name="warmps")
            for r in range(2):
                nc.tensor.matmul(wps[:1, :], wsrc[:, :1], wsrc[:],
                                 start=(r == 0), stop=(r == 1))

            ones_f = constp.tile([128, 1], F32, tag="ones32")
            nc.gpsimd.memset(ones_f[:], 1.0)
            ones = constp.tile([128, 1], BF16, tag="ones")
            nc.vector.tensor_copy(ones[:], ones_f[:])
            # multiplicative diag pattern p: [128k, 512q] bf16,
            # 1 where q >= k + 128p else 0 (keep-mask, applied post-exp)
            diag = []
            for p in range(4):
                dm = constp.tile([128, 512], BF16, tag=f"diag{p}")
                nc.gpsimd.memset(dm[:], 1.0)
                nc.gpsimd.affine_select(
                    out=dm[:], in_=dm[:],
                    compare_op=mybir.AluOpType.is_ge,
                    fill=0.0, base=-128 * p,
                    pattern=[[1, 512]], channel_multiplier=-1,
                )
                diag.append(dm)

            warm = constp.tile([1, 1], F32, tag="warm")
            nc.scalar.activation(
                warm[:], ones_f[:1, :1], mybir.ActivationFunctionType.Exp, scale=1.0
            )
            nc.sync.dma_start(warm_out, warm[:])

            # ---- projection targets ----
            # Q/K in fp8 DoubleRow layout [64p, 2, cols]: h = 64*g + p for
            # DR group g - the S matmul contracts (p, g) pairs at 0.5
            # cycles/row. V [k-tiles, h] stays bf16 (fp8 V fails tolerance).
            # kv tile space: tiles 0..7 = rect-k, 8..23 = tri chunk
            qt = projp.tile([64, 2, TQ], FP8, tag="qt")
            kt = projp.tile([64, 2, RK + CH], FP8, tag="kt")
            vsb = projp.tile([128, NKT_RK + CH // 128, 128], BF16, tag="v")
            l_sb = projp.tile([1, TQ], F32, tag="lsb")

            xblocks = {("q", 0): (xb0a, xb0b)}

            def get_xb(src, key, tb):
                """x block loaded as two half tiles, each in two quarter
                DMAs - small transfers so the fp8-shift DMAs are never
                stuck behind a long one."""
                if key in xblocks:
                    return xblocks[key]
                xa = xin.tile([128, 2, 2, 2, 512], FP8, tag="xb")
                nc.sync.dma_start(xa[:], src[:, tb, :2])
                xb = xin.tile([128, 2, 2, 2, 512], FP8, tag="xb")
                nc.sync.dma_start(xb[:], src[:, tb, 2:])
                xblocks[key] = (xa, xb)
                return xblocks[key]

            _stg_n = [0]

            def stg_alt():
                _stg_n[0] += 1
                return _stg_n[0] % 2 == 0

            # 3-term residual fp8 projection: x@W ~= x8@W8 + xr8@W8 + x8@Wr8
            # (residuals in e5m2; the dropped xr8@Wr8 term is ~0.2%)
            TERMS = ((0, 0), (1, 0), (0, 1))  # (x-sel, w-sel)

            def xsl(xpair, h, pi, xs):
                """[128, 2(g), 512] x slice: ecp = 2h + pi."""
                a = xpair[h][:, pi, xs]
                return a.bitcast(FP8E5) if xs else a

            def wsl(name, ws, ecp):
                """[128, 2(g), H] weight slice."""
                a = wt[:, NI[name], ws, ecp]
                return a.bitcast(FP8E5) if ws else a

            # proj work is emitted in half-units so it can be woven
            # finely between attention jobs
            def proj_qk(name, xpair, dcol):
                """One 512-token q/k projection (3-term fp8 DR) as two
                half-thunks (ecp 0-1 / 2-3, 6 DR matmuls each).

                Evacuation to the fp8 DR layout: h 0..63 copies straight
                (same partitions); h 64..127 converts into a staging tile
                and a small SBUF->SBUF DMA shifts it down to partitions
                0..63, group 1; the two half-conversions run on different
                engines in parallel so the psum slot frees sooner.
                """
                dst = qt if name == "q" else kt
                ps = psT.tile([128, 512], F32, tag="psproj")

                def half(h):
                    # Q/K drop the x8@Wr8 term (2-term): the W-quantization
                    # noise lands on S logits where the error budget has
                    # margin; V keeps all 3 terms (it is tolerance-critical)
                    terms = TERMS[:2]
                    for ti, (xs, ws) in enumerate(terms):
                        for pi in range(2):
                            nc.tensor.matmul(
                                ps[:], wsl(name, ws, 2 * h + pi),
                                xsl(xpair, h, pi, xs),
                                start=(h == 0 and ti == 0 and pi == 0),
                                stop=(h == 1 and ti == len(terms) - 1
                                      and pi == 1),
                                perf_mode=DR,
                            )
                    if h == 1:
                        nc.vector.tensor_copy(
                            dst[:, 0, dcol : dcol + 512], ps[:64, :]
                        )
                        stg = stgp.tile([128, 512], FP8, tag="stg")
                        nc.vector.tensor_copy(stg[64:, :], ps[64:, :])
                        nc.gpsimd.tensor_copy(
                            dst[:, 1, dcol : dcol + 512], stg[64:, :]
                        )

                yield lambda: half(0)
                yield lambda: half(1)

            def proj_v(xpair, kv0):
                """One 512-token V projection directly as [tok, h] tiles,
                3-term fp8 DR, as two half-thunks (2 token-tiles each).

                x slice is stationary, Wv is moving: out[tok, h] per
                128-token tile; 4 tiles side by side in one psum bank,
                one evac into vsb[:, kv0:kv0+4, :].
                """
                ps = psT.tile([128, 512], F32, tag="psproj")

                def half(hh):
                    for t in range(2 * hh, 2 * hh + 2):
                        dst = ps[:, 128 * t : 128 * (t + 1)]
                        mm = 0
                        for ti, (xs, ws) in enumerate(TERMS):
                            for h in range(2):
                                for pi in range(2):
                                    nc.tensor.matmul(
                                        dst,
                                        xsl(xpair, h, pi, xs)[
                                            :, :, 128 * t : 128 * (t + 1)],
                                        wsl("v", ws, 2 * h + pi),
                                        start=(mm == 0), stop=(mm == 11),
                                        perf_mode=DR,
                                    )
                                    mm += 1
                    if hh == 1:
                        # V evac on Act: it is idle during the proj-heavy
                        # phase while DVE is the loaded engine there
                        nc.scalar.copy(
                            vsb[:, kv0 // 128 : kv0 // 128 + 4, :], ps[:]
                        )

                yield lambda: half(0)
                yield lambda: half(1)

            def thunk_kvq(b):
                def f():
                    xp = get_xb(xq_in, ("q", b), b)
                    yield from proj_qk("k", xp, RK + 512 * b)
                    yield from proj_qk("q", xp, 512 * b)
                    yield from proj_v(xp, RK + 512 * b)
                return f

            def thunk_kq(b):
                def f():
                    xp = get_xb(xq_in, ("q", b), b)
                    yield from proj_qk("k", xp, RK + 512 * b)
                    yield from proj_qk("q", xp, 512 * b)
                return f

            def thunk_vq(b):
                def f():
                    xp = get_xb(xq_in, ("q", b), b)
                    yield from proj_v(xp, RK + 512 * b)
                return f

            def thunk_rk(b):
                def f():
                    xp = get_xb(xrk_in, ("rk", b), b)
                    yield from proj_qk("k", xp, 512 * b)
                    yield from proj_v(xp, 512 * b)
                return f

            def thunk_q(b):
                def f():
                    xp = get_xb(xq_in, ("q", b), b)
                    yield from proj_qk("q", xp, 512 * b)
                return f

            # ---- attention job list (same folded-triangle pairing as v1) ----
            # job = (qb, (k0, d0), (k1, d1), first_in_block, last_in_block)
            # d = diag position (0..3) for tiles on the causal diagonal; the
            # AV matmul for such tiles only covers q-columns [128*d, 512).
            jobs = []
            for qb in range(4):  # triangle over kv tiles 8..(8+4qb+4)
                kts = []
                for j in range(4 * qb + 4):
                    dp = j - 4 * qb if j >= 4 * qb else None
                    kts.append((NKT_RK + j, dp))
                for i in range(len(kts) // 2):
                    jobs.append((qb, kts[2 * i], kts[2 * i + 1],
                                 i == 0, 2 * i + 2 == len(kts)))
            for qb in range(4, 8):  # rect over kv tiles 0..7
                for i in range(NKT_RK // 2):
                    jobs.append((qb, (2 * i, None), (2 * i + 1, None),
                                 i == 0, 2 * i + 2 == NKT_RK))

            n = len(jobs)
            ss_t = [None] * n
            pt_t = [None] * n
            ybank = {}
            pacc = {}
            pending_out = []

            def s_stage(j):
                qb, (k0, d0), (k1, d1), _, _ = jobs[j]
                ss = psS.tile([128, 1024], F32, tag="s")
                ss_t[j] = ss
                qs = qt[:, :, 512 * qb : 512 * (qb + 1)]
                nc.tensor.matmul(ss[:, :512], kt[:, :, 128 * k0 : 128 * (k0 + 1)],
                                 qs, start=True, stop=True,
                                 perf_mode=mybir.MatmulPerfMode.DoubleRow)
                nc.tensor.matmul(ss[:, 512:], kt[:, :, 128 * k1 : 128 * (k1 + 1)],
                                 qs, start=True, stop=True,
                                 perf_mode=mybir.MatmulPerfMode.DoubleRow)
                pt = ppool.tile([128, 1024], BF16, tag="pt")
                pt_t[j] = pt
                nc.scalar.activation(
                    pt[:], ss[:], mybir.ActivationFunctionType.Exp, scale=SCALE
                )

            def av_stage(j):
                qb, (k0, d0), (k1, d1), first, last = jobs[j]
                pt = pt_t[j]
                # causal mask on the diagonal tiles: zero the upper-triangle
                # region of pt in place on the (otherwise idle) Pool engine,
                # keeping DVE free for the P-accumulation
                if d0 is not None:
                    nc.gpsimd.affine_select(
                        out=pt[:, :512], in_=pt[:, :512],
                        compare_op=mybir.AluOpType.is_ge,
                        fill=0.0, base=-128 * d0,
                        pattern=[[1, 512]], channel_multiplier=-1,
                    )
                if d1 is not None:
                    nc.gpsimd.affine_select(
                        out=pt[:, 512:], in_=pt[:, 512:],
                        compare_op=mybir.AluOpType.is_ge,
                        fill=0.0, base=-128 * d1,
                        pattern=[[1, 512]], channel_multiplier=-1,
                    )
                if first:
                    ys = psY.tile([128, 512], F32, tag="y", name=f"ys{qb}")
                    ybank[qb] = ys
                else:
                    ys = ybank[qb]
                # diagonal tiles only contribute to q >= 128*d: shrink the
                # AV matmul to the live columns (mask already zeroed the rest
                # of pt for the P-accumulation)
                o0 = 128 * d0 if d0 else 0
                o1 = 128 * d1 if d1 else 0
                nc.tensor.matmul(ys[:, o0:], vsb[:, k0, :], pt[:, o0:512],
                                 start=first, stop=False)
                nc.tensor.matmul(ys[:, o1:], vsb[:, k1, :], pt[:, 512 + o1 :],
                                 start=False, stop=last)
                if last:
                    pending_out.append(qb)

            def pacc_ops(qb):
                """P-accumulation ops for a finished q-block, deferred to
                the next window: all pt inputs are long done, so these DVE
                adds never block the in-order DVE queue (whose later entries
                feed the PE's projection pipeline). Returned as thunks so
                the window can spread them out instead of bursting."""
                ops = []

                def first(j):
                    pa = paccp.tile([128, 512], BF16, tag="pacc",
                                    name=f"pacc{qb}")
                    pacc[qb] = pa
                    nc.vector.tensor_add(pa[:], pt_t[j][:, :512],
                                         pt_t[j][:, 512:])

                def rest(j, half):
                    pa = pacc[qb]
                    nc.vector.tensor_add(pa[:], pa[:],
                                         pt_t[j][:, 512 * half : 512 * half + 512])

                for i, j in enumerate(by_qb[qb]):
                    if i == 0:
                        ops.append(lambda j=j: first(j))
                    else:
                        ops.append(lambda j=j: rest(j, 0))
                        ops.append(lambda j=j: rest(j, 1))
                return ops

            def flush_out():
                """Row-sum + output DMA for a finished q-block. Deferred to
                the next window so the PE's ones-matmul never waits on the
                tail of the DVE P-accumulation chain; outputs DMA straight
                from PSUM (no DVE evacuation)."""
                while pending_out:
                    qb = pending_out.pop(0)
                    ys, pa = ybank[qb], pacc[qb]
                    ls = psT.tile([128, 512], F32, tag="psproj", name=f"ls{qb}")
                    nc.tensor.matmul(ls[:1, :], ones[:], pa[:],
                                     start=True, stop=True)
                    yo = outp.tile([128, 512], F32, tag="yo")
                    # the very last q-block evacuates on Act (idle by then,
                    # and parallel with DVE's inline P-accumulation tail)
                    cp = nc.scalar.copy if qb == worder[-1] else \
                        nc.vector.tensor_copy
                    cp(yo[:], ys[:])
                    nc.sync.dma_start(acc_out[:, 512 * qb : 512 * (qb + 1)], yo[:])
                    # l rows collect in SBUF; two DMAs total (the HWDGE
                    # queue charges a fixed cost per DMA)
                    cp(l_sb[:, 512 * qb : 512 * (qb + 1)], ls[:1, :])

            # ---- interleaved schedule ----
            # per-qb window: list of proj thunks to weave between that
            # window's jobs (deps: kv(b) ready before qb=b jobs; rk before
            # qb4; q(b) before qb=b / qb=4+b jobs)
            # window order interleaves rect q-blocks into the proj-heavy tri
            # phase: rect jobs are exp-chain-bound, tri windows are PE-bound
            # with Act idle - blending them keeps both engines fed
            worder = [0, 1, 4, 2, 5, 3, 6, 7]
            pre = [thunk_kvq(0)]
            weave = {
                0: [thunk_kvq(1)],
                1: [thunk_rk(0), thunk_rk(1), thunk_q(4)],
                4: [thunk_kq(2)],
                2: [thunk_vq(2), thunk_q(5)],
                5: [thunk_kq(3)],
                3: [thunk_vq(3), thunk_q(6)],
                6: [thunk_q(7)],
                7: [],
            }

            def expand(thunks):
                ops = []
                for t in thunks:
                    ops.extend(list(t()))
                return ops

            # pre-phase: k/q of block 0, then the first two S stages
            # (they do not need V), then V - the exp chain starts ~1.7us
            # earlier
            pre_ops = expand(pre)
            for op in pre_ops[:4]:
                op()

            # group jobs by qb; execution sequence follows worder
            by_qb = {}
            for j, jb in enumerate(jobs):
                by_qb.setdefault(jb[0], []).append(j)
            jseq = [j for qb in worder for j in by_qb[qb]]
            pos_of = {j: i for i, j in enumerate(jseq)}

            def emit_window(qb, prev_qb, inline_pacc=False):
                js = by_qb[qb]
                ops = expand(weave[qb])
                nj = len(js)
                done = 0
                pac = pacc_ops(prev_qb) if prev_qb is not None else []
                pdone = 0
                for i, j in enumerate(js):
                    # weave proj ops evenly, finishing early: the
                    # lookahead-2 S of the next window needs them. Proj ops
                    # go BEFORE the pacc adds so their DVE evacuations are
                    # never stuck behind the (never-blocking but bulky)
                    # accumulation queue.
                    want = len(ops)
                    while done < want:
                        ops[done]()
                        done += 1
                    # spread the previous q-block's P-accumulation adds
                    # across the window; all done before the flush at the
                    # last iteration (psY liveness allows that deferral)
                    pwant = min(len(pac), (i + 2) * len(pac) // nj)
                    while pdone < pwant:
                        pac[pdone]()
                        pdone += 1
                    if i == nj - 1:
                        flush_out()  # previous q-block's row-sum + out DMA
                    # issue S two jobs ahead so an exp-lagged AV never
                    # leaves the PE without queued work
                    p2 = pos_of[j] + 2
                    if p2 < len(jseq):
                        s_stage(jseq[p2])
                    av_stage(j)
                    if inline_pacc:
                        # last window: accumulate as we go so the program
                        # tail is not serialized behind a deferred chain
                        pt = pt_t[j]
                        if i == 0:
                            pa = paccp.tile([128, 512], BF16, tag="pacc",
                                            name=f"pacc{qb}")
                            pacc[qb] = pa
                            nc.vector.tensor_add(pa[:], pt[:, :512], pt[:, 512:])
                        else:
                            pa = pacc[qb]
                            nc.vector.tensor_add(pa[:], pa[:], pt[:, :512])
                            nc.vector.tensor_add(pa[:], pa[:], pt[:, 512:])

            s_stage(jseq[0])
            s_stage(jseq[1])
            for op in pre_ops[4:]:
                op()
            prev = None
            for wi, qb in enumerate(worder):
                emit_window(qb, prev, inline_pacc=(wi == len(worder) - 1))
                if wi == len(worder) - 2:
                    # ship the first 6 q-blocks' row-sums before the tail
                    nc.sync.dma_start(l_out[:, : 512 * 6],
                                      l_sb[:, : 512 * 6])
                prev = qb
            flush_out()
            nc.sync.dma_start(l_out[:, 512 * 6 :], l_sb[:, 512 * 6 :])

    nc.compile()
    return nc


def _prep_x(xpart):
    """[Tpart, E] f32 -> fp8 3-term layout [128, tb, ecp, sel, g, 512].

    e-index = 256*ecp + 128*g + p. sel 0 = e4m3(x), sel 1 = e5m2(x - x8)
    (e5m2 bits shipped in the e4m3 container; the device bitcasts).
    """
    tb = xpart.shape[0] // 512
    a = xpart.T.astype(np.float32).reshape(4, 2, 128, tb, 512)
    x8 = np.asarray(a, dtype=ml_dtypes.float8_e4m3)
    xr8 = np.asarray(a - np.asarray(x8, np.float32),
                     dtype=ml_dtypes.float8_e5m2)
    out = np.empty((128, tb, 4, 2, 2, 512), np.uint8)
    out[:, :, :, 0] = x8.view(np.uint8).transpose(2, 3, 0, 1, 4)
    out[:, :, :, 1] = xr8.view(np.uint8).transpose(2, 3, 0, 1, 4)
    return np.ascontiguousarray(out).view(ml_dtypes.float8_e4m3)


def _prep_w(w):
    """[H, E] f32 -> fp8 [128, sel, ecp, g, H] (W8 + e5m2 residual)."""
    a = w.T.astype(np.float32).reshape(4, 2, 128, H)
    w8 = np.asarray(a, dtype=ml_dtypes.float8_e4m3)
    wr8 = np.asarray(a - np.asarray(w8, np.float32),
                     dtype=ml_dtypes.float8_e5m2)
    out = np.empty((128, 2, 4, 2, H), np.uint8)
    out[:, 0] = w8.view(np.uint8).transpose(2, 0, 1, 3)
    out[:, 1] = wr8.view(np.uint8).transpose(2, 0, 1, 3)
    return out


def kernel(x_in, Wq, Wk, Wv):
    B, T_, E_ = x_in.shape
    assert (B, T_, E_) == (4, T, E)
    nc = _CACHED.get("nc")
    if nc is None:
        nc = _CACHED["nc"] = _build()

    # weights in one tensor, order (k, q, v) to match the device layout
    w = np.ascontiguousarray(
        np.stack([_prep_w(Wk), _prep_w(Wq), _prep_w(Wv)], axis=1)
    ).view(ml_dtypes.float8_e4m3)
    in_maps = []
    for c in range(8):
        b, h = c // 2, c % 2
        xb = np.asarray(x_in[b], dtype=np.float32)
        c0, c1 = xb[:CH], xb[CH:]
        own = c0 if h == 0 else c1
        xq = np.concatenate([own, c1], axis=0)        # [4096, E]
        rk = xb[0:RK] if h == 0 else xb[RK : 2 * RK]  # [1024, E]
        in_maps.append(
            {"xq_in": _prep_x(xq), "xrk_in": _prep_x(rk), "w_in": w}
        )

    kw = {}
    if TRACE:
        kw = {"trace": True, "trace_cores": TRACE_CORES}
    res = run_bass_kernel_spmd(nc, in_maps, core_ids=list(range(8)), **kw)
    global LAST_RESULTS
    LAST_RESULTS = res

    y = np.empty((B, T, H), dtype=np.float32)
    for b in range(4):
        r0, r1 = res.results[2 * b], res.results[2 * b + 1]
        a0, l0 = r0["acc_out"], r0["l_out"][0]
        a1, l1 = r1["acc_out"], r1["l_out"][0]
        y[b, :CH] = (a0[:, :CH] / l0[:CH]).T
        acc = a0[:, CH:] + a1[:, :CH] + a1[:, CH:]
        l = l0[CH:] + l1[:CH] + l1[CH:]
        y[b, CH:] = (acc / l).T
    return y
